# revision 13
# baseline (speedup 1.0000x reference)
"""Causal attention head on 8 TRN2 NeuronCores, data-parallel over batch.

Per-core (one batch element, S=2048, D=1024):
  QT = (Wq^T @ Xq^T + bq^T) / 32          [f, q]   (f32r)
  KT = Wk^T @ Xk^T + bk^T                  [f, k]   (f32r)
  V  = Xv^T.T @ Wv + 1 x bv                [k, f]   (f32r)
  S  = QT.T @ KT  (+ mask on diag chunk)   per 128-q-block, 512-key chunks
  P  = exp(S) (no max subtraction; scores are O(5), exp(-1e9)=0 underflow)
  l  = row-sum(P);  O = (P @ V) * (1/l)
X^T via PE transposes (identity matmul). All matmul operands rounded to
float32r (TF32-like, 4x faster than fp32 on the PE at moving dim >= 256).
"""
import numpy as np

S = 2048
D = 1024
B = 8
NQB = S // 128   # 16 query blocks
NKC = S // 512   # 4 key chunks
SCALE = float(1.0 / np.sqrt(D))

_CACHE = {}


def _build(causal: bool, use_f32r: bool):
    import concourse.bass as bass
    import concourse.mybir as mybir
    import concourse.tile as tile
    from concourse import bacc
    from concourse.masks import make_identity

    mdt = mybir.dt.float32r if use_f32r else mybir.dt.float32
    f32 = mybir.dt.float32
    Exp = mybir.ActivationFunctionType.Exp
    Ident = mybir.ActivationFunctionType.Identity

    nc = bacc.Bacc("TRN2", target_bir_lowering=False, debug=False)
    q_d = nc.dram_tensor("query", [S, D], f32, kind="ExternalInput").ap()
    k_d = nc.dram_tensor("key", [S, D], f32, kind="ExternalInput").ap()
    v_d = nc.dram_tensor("value", [S, D], f32, kind="ExternalInput").ap()
    wq_d = nc.dram_tensor("wq", [D, D], f32, kind="ExternalInput").ap()
    wk_d = nc.dram_tensor("wk", [D, D], f32, kind="ExternalInput").ap()
    wv_d = nc.dram_tensor("wv", [D, D], f32, kind="ExternalInput").ap()
    # bqt is pre-scaled by 1/32 on host; layout [128, 8]: bqt[p, t] = bq[t*128+p]
    bqt_d = nc.dram_tensor("bqt", [128, 8], f32, kind="ExternalInput").ap()
    bkt_d = nc.dram_tensor("bkt", [128, 8], f32, kind="ExternalInput").ap()
    bvr_d = nc.dram_tensor("bvr", [1, D], f32, kind="ExternalInput").ap()
    if causal:
        mask_d = nc.dram_tensor("maskd", [NQB, 128, 512], f32, kind="ExternalInput").ap()
    else:
        mask_d = nc.dram_tensor("maskf", [S, S], f32, kind="ExternalInput").ap()
    out_d = nc.dram_tensor("out", [S, D], f32, kind="ExternalOutput").ap()

    with tile.TileContext(nc) as tc:
        with (
            tc.tile_pool(name="big", bufs=8) as big,       # KT tiles
            tc.tile_pool(name="vpool", bufs=16) as vpool,  # V tiles
            tc.tile_pool(name="wpool", bufs=8) as wpool,   # Wk -> Wv -> Wq
            tc.tile_pool(name="xt", bufs=8) as xtp,        # X^T slices + P^T chunks
            tc.tile_pool(name="qt", bufs=8) as qtp,        # QT group tiles
            tc.tile_pool(name="xnat", bufs=3) as xnat,     # natural X row-tiles (half rows)
            tc.tile_pool(name="pp", bufs=4) as pp,         # P row chunks
            tc.tile_pool(name="mk", bufs=1) as mk,         # mask chunks
            tc.tile_pool(name="ob", bufs=2) as ob,         # output staging
            tc.tile_pool(name="small", bufs=1) as small,
            tc.tile_pool(name="stats", bufs=3) as stats,
            tc.tile_pool(name="ps_tr", bufs=2, space="PSUM") as ps_tr,
            tc.tile_pool(name="ps_pj", bufs=2, space="PSUM") as ps_pj,
            tc.tile_pool(name="ps_s", bufs=2, space="PSUM") as ps_s,
            tc.tile_pool(name="ps_o", bufs=2, space="PSUM") as ps_o,
        ):
            ident = small.tile([128, 128], f32, tag="ident")
            make_identity(nc, ident)

            bqt = small.tile([128, 8], f32, tag="bqt")
            nc.sync.dma_start(out=bqt, in_=bqt_d)
            bkt = small.tile([128, 8], f32, tag="bkt")
            nc.sync.dma_start(out=bkt, in_=bkt_d)
            # bv halves at partitions 0 and 64 (matmul base-partition rule)
            bvr = small.tile([128, 512], mdt, tag="bvr")
            nc.gpsimd.dma_start(out=bvr[0:1, :], in_=bvr_d[0:1, 0:512])
            nc.gpsimd.dma_start(out=bvr[64:65, :], in_=bvr_d[0:1, 512:1024])
            ones_f = xnat.tile([128, 128], f32, tag="xnat")
            nc.vector.memset(ones_f, 1.0)
            ones_k = small.tile([128, 128], mdt, tag="ones_k")
            nc.scalar.copy(ones_k, ones_f)

            def load_w(w_dram):
                tiles = []
                for dj in range(8):
                    t = wpool.tile([128, D], mdt, tag="w")
                    nc.gpsimd.dma_start(out=t, in_=w_dram[dj * 128:(dj + 1) * 128, :])
                    tiles.append(t)
                return tiles

            def transpose_rows(x_dram, row0, nrow_tiles, width):
                """Load nrow_tiles x [128, D] rows of x and return xT as 8
                tiles [128 (d-slice), width] in mdt (width = nrow_tiles*128)."""
                xT = [xtp.tile([128, width], mdt, tag="xt", name=f"xT{i}") for i in range(8)]
                for t in range(nrow_tiles):
                    r = row0 + t * 128
                    for half in range(2):
                        nat = xnat.tile([128, 512], f32, tag="xnat")
                        nc.sync.dma_start(
                            out=nat, in_=x_dram[r:r + 128, half * 512:(half + 1) * 512]
                        )
                        ps = ps_tr.tile([128, 512], f32, tag="tr")
                        for j in range(4):
                            nc.tensor.transpose(
                                ps[:, j * 128:(j + 1) * 128],
                                nat[:, j * 128:(j + 1) * 128],
                                ident,
                            )
                        for j in range(4):
                            dj = half * 4 + j
                            nc.vector.tensor_copy(
                                xT[dj][:, t * 128:(t + 1) * 128],
                                ps[:, j * 128:(j + 1) * 128],
                            )
                return xT

            # ---- KT = Wk^T @ Xk^T + bk ----
            wk = load_w(wk_d)
            kt_tiles = [big.tile([128, S], mdt, tag="kt", name=f"kt{i}") for i in range(8)]
            for kc in range(NKC):
                xkT = transpose_rows(k_d, kc * 512, 4, 512)
                for fi in range(8):
                    ps = ps_pj.tile([128, 512], f32, tag="pj")
                    for dj in range(8):
                        nc.tensor.matmul(
                            ps, wk[dj][:, fi * 128:(fi + 1) * 128], xkT[dj],
                            start=(dj == 0), stop=(dj == 7),
                        )
                    nc.scalar.activation(
                        kt_tiles[fi][:, kc * 512:(kc + 1) * 512], ps, Ident,
                        bias=bkt[:, fi:fi + 1], scale=1.0,
                    )

            # ---- V = Xv @ Wv + bv ----
            wv = load_w(wv_d)
            v_tiles = [vpool.tile([128, D], mdt, tag="v", name=f"v{i}") for i in range(NQB)]
            for kc in range(NKC):
                xvT = transpose_rows(v_d, kc * 512, 4, 512)
                for kt in range(4):
                    for fc in range(2):
                        ps = ps_pj.tile([128, 512], f32, tag="pj")
                        for dj in range(8):
                            nc.tensor.matmul(
                                ps, xvT[dj][:, kt * 128:(kt + 1) * 128],
                                wv[dj][:, fc * 512:(fc + 1) * 512],
                                start=(dj == 0), stop=False,
                            )
                        p0 = 64 * fc
                        nc.tensor.matmul(
                            ps, ones_k[p0:p0 + 1, :], bvr[p0:p0 + 1, :],
                            start=False, stop=True,
                        )
                        nc.scalar.copy(
                            v_tiles[kc * 4 + kt][:, fc * 512:(fc + 1) * 512], ps,
                        )

            # ---- attention, 2 q-blocks (256 rows) per group ----
            wq = load_w(wq_d)
            for g in range(NQB // 2):
                xqT = transpose_rows(q_d, g * 256, 2, 256)
                qtg = []
                for fi in range(8):
                    ps = ps_pj.tile([128, 256], f32, tag="pj")
                    for dj in range(8):
                        nc.tensor.matmul(
                            ps, wq[dj][:, fi * 128:(fi + 1) * 128], xqT[dj],
                            start=(dj == 0), stop=(dj == 7),
                        )
                    qt = qtp.tile([128, 256], mdt, tag="qt")
                    nc.scalar.activation(
                        qt, ps, Ident, bias=bqt[:, fi:fi + 1], scale=SCALE,
                    )
                    qtg.append(qt)

                for qb in range(2):
                    qi = g * 2 + qb
                    nk = qi + 1 if causal else NQB          # causal kj blocks
                    nch = (nk + 3) // 4                      # 512-wide chunks
                    lsum = stats.tile([128, 4], f32, tag="lsum")
                    p_chunks = []
                    for c in range(nch):
                        ps = ps_s.tile([128, 512], f32, tag="s")
                        for fi in range(8):
                            nc.tensor.matmul(
                                ps, qtg[fi][:, qb * 128:(qb + 1) * 128],
                                kt_tiles[fi][:, c * 512:(c + 1) * 512],
                                start=(fi == 0), stop=(fi == 7),
                            )
                        if causal:
                            diag = (c == nch - 1)
                        else:
                            diag = True
                        if diag:
                            m = mk.tile([128, 512], f32, tag="m")
                            if causal:
                                nc.sync.dma_start(out=m, in_=mask_d[qi])
                            else:
                                nc.sync.dma_start(
                                    out=m,
                                    in_=mask_d[qi * 128:(qi + 1) * 128,
                                               c * 512:(c + 1) * 512],
                                )
                            nc.vector.tensor_add(ps, ps, m)
                        pc = pp.tile([128, 512], f32, tag="p")
                        nc.scalar.activation(
                            pc, ps, Exp, bias=0.0, scale=1.0,
                            accum_out=lsum[:, c:c + 1],
                        )
                        p_chunks.append(pc)

                    l_tot = stats.tile([128, 1], f32, tag="l")
                    nc.vector.reduce_sum(
                        out=l_tot, in_=lsum[:, :nch], axis=mybir.AxisListType.X,
                    )
                    inv = stats.tile([128, 1], f32, tag="inv")
                    nc.vector.reciprocal(inv, l_tot)

                    # transpose P -> pT chunks (f32r)
                    pT = []
                    for c in range(nch):
                        nblk = min(4, nk - c * 4)
                        ps = ps_tr.tile([128, 512], f32, tag="tr")
                        for j in range(nblk):
                            nc.tensor.transpose(
                                ps[:, j * 128:(j + 1) * 128],
                                p_chunks[c][:, j * 128:(j + 1) * 128],
                                ident,
                            )
                        pt = xtp.tile([128, 512], mdt, tag="xt")
                        nc.scalar.copy(
                            pt[:, :nblk * 128], ps[:, :nblk * 128],
                        )
                        pT.append(pt)

                    for fc in range(2):
                        ps = ps_o.tile([128, 512], f32, tag="o")
                        for kj in range(nk):
                            nc.tensor.matmul(
                                ps, pT[kj // 4][:, (kj % 4) * 128:(kj % 4 + 1) * 128],
                                v_tiles[kj][:, fc * 512:(fc + 1) * 512],
                                start=(kj == 0), stop=(kj == nk - 1),
                            )
                        o_sb = ob.tile([128, 512], f32, tag="osb")
                        nc.vector.tensor_scalar_mul(o_sb, ps, inv)
                        nc.sync.dma_start(
                            out=out_d[qi * 128:(qi + 1) * 128,
                                      fc * 512:(fc + 1) * 512],
                            in_=o_sb,
                        )

    nc.compile()
    return nc


def _get_nc(causal: bool, use_f32r: bool = True):
    key = (causal, use_f32r)
    if key not in _CACHE:
        _CACHE[key] = _build(causal, use_f32r)
    return _CACHE[key]


def _is_causal(mask):
    exp = np.triu(np.full((S, S), -1e9, dtype=np.float32), k=1)
    return mask.shape == (1, S, S) and np.array_equal(np.asarray(mask)[0], exp)


def kernel(query, key, value, mask, Wq, bq, Wk, bk, Wv, bv):
    from concourse.bass_utils import run_bass_kernel_spmd

    query = np.ascontiguousarray(np.asarray(query, dtype=np.float32))
    key = np.ascontiguousarray(np.asarray(key, dtype=np.float32))
    value = np.ascontiguousarray(np.asarray(value, dtype=np.float32))
    mask = np.asarray(mask, dtype=np.float32)

    causal = _is_causal(mask)
    nc = _get_nc(causal)

    def btile(b):  # [128, 8] layout: bt[p, t] = b[t*128 + p]
        return np.ascontiguousarray(np.asarray(b, np.float32).reshape(8, 128).T)

    shared = {
        "wq": np.ascontiguousarray(np.asarray(Wq, np.float32)),
        "wk": np.ascontiguousarray(np.asarray(Wk, np.float32)),
        "wv": np.ascontiguousarray(np.asarray(Wv, np.float32)),
        "bqt": btile(np.asarray(bq, np.float32) * SCALE),
        "bkt": btile(bk),
        "bvr": np.ascontiguousarray(np.asarray(bv, np.float32).reshape(1, D)),
    }

    if causal:
        m0 = mask[0]
        md = np.stack([
            m0[qi * 128:(qi + 1) * 128, (qi // 4) * 512:(qi // 4 + 1) * 512]
            for qi in range(NQB)
        ])
        shared["maskd"] = np.ascontiguousarray(md)
    else:
        shared["maskf"] = np.ascontiguousarray(mask[0])

    in_maps = [
        {"query": query[b], "key": key[b], "value": value[b], **shared}
        for b in range(B)
    ]
    res = run_bass_kernel_spmd(nc, in_maps, list(range(B)))
    return np.stack([res.results[b]["out"] for b in range(B)])
